# revision 4
# baseline (speedup 1.0000x reference)
"""GAT encoder (gnn_message_passing) on 8 trn2 NeuronCores via Bass.

Strategy (graph-parallel, dst-sharded):
  Launch 1 (sharded by node range): h = x@W1, es = x@(W1@att_src),
    ed = x@(W1@att_dst). Each core writes haug rows [h fp32 | es fp32]
    (129 floats = 516B) for its 6250 nodes, plus ed.
  Host: concatenate haug shards (+1 dummy row with es=-1e30), permute ed
    into degree-sorted window layout, route edges to dst-owner cores.
  Launch 2 (per core, dst windows of 128 degree-sorted nodes): indirect-DMA
    gather of haug rows for all edge slots (padded to per-window uniform
    chunk count); alpha = sigmoid(es_src + ed_dst); ex = exp(alpha)
    (max-subtraction dropped: alpha in (0,1) so exp is stable; softmax
    weights are mathematically identical); per-chunk scale rows by ex and
    accumulate via identity-stationary matmuls into PSUM; denominator =
    free-dim reduce of ex minus pad count; out = ELU(acc/den) @ W2.
"""
import os
import sys
import time

sys.path.insert(0, "/opt/trn_rl_repo")

import numpy as np

N, E = 50000, 800000
IN, HID, OUT = 256, 128, 128
NCORES = 8
NPC = N // NCORES            # nodes per core
NW = NPC // 128              # windows per core (49 when NPC=6272... 6250/128)
assert NPC % 2 == 0
NW = (NPC + 127) // 128      # 49 windows; last window partial (6250 = 48*128+106)
ROWF = HID + 1               # floats per haug row (h + es)
DUMMY = N                    # dummy haug row index (es = -1e30)
GCOLS = 64                   # max idx columns per gather call

_timings = {}


def _patch_env():
    """Tile/perfetto compatibility patches for this container."""
    import concourse.tile as tile
    from concourse.tile import ScopedClock

    def _drain_and_barrier_split(self, tick_clock, wait_clock):
        nc = self.nc
        probe = nc.sync.nop()
        wait_clock.add_sem_waits(
            probe.ins, ScopedClock({None: tick_clock.global_clock})
        )
        waits = list(probe.ins.sync_info.on_wait or [])
        probe.ins.sync_info.on_wait = []
        from concourse import mybir

        for w in waits:
            inst = nc.sync.nop()
            if inst.ins.sync_info is None:
                inst.ins.sync_info = mybir.SyncInfo(on_wait=[w], on_update=[])
            else:
                inst.ins.sync_info.on_wait = [w]
        nc.sync.drain()
        nc.all_engine_barrier()
        assert self.sems is not None
        popped = nc._tile_sem_poison_stack.pop()
        assert popped is self._sem_poison
        nc.clear_and_free_semaphores(list(self.sems.allocated().values()))
        nc.all_engine_barrier()

    tile.TileContext._drain_and_barrier = _drain_and_barrier_split


_patch_env()


def _patch_perfetto():
    try:
        from gauge import trn_perfetto

        cls = trn_perfetto.TrnPerfettoConv
        if not getattr(cls, "_no_hlo_patched", False):
            _orig_init = cls.__init__

            def _init_no_hlo(self, *a, **k):
                k["annotate_hlo"] = False
                if len(a) >= 2:
                    a = (a[0], False) + a[2:]
                _orig_init(self, *a, **k)

            cls.__init__ = _init_no_hlo
            cls._no_hlo_patched = True
    except Exception:
        pass


import concourse.bass as bass
import concourse.bacc as bacc
import concourse.tile as tile
from concourse import mybir
from concourse.bass_utils import run_bass_kernel_spmd
from concourse.masks import make_identity

F32 = mybir.dt.float32
I32 = mybir.dt.int32
AF = mybir.ActivationFunctionType
ALU = mybir.AluOpType


# ---------------------------------------------------------------- phase 1
def build_phase1():
    nc = bacc.Bacc("TRN2", target_bir_lowering=True)
    ntiles = (NPC + 127) // 128
    npad = ntiles * 128
    xT = nc.dram_tensor("xT", [IN, npad], F32, kind="ExternalInput")
    w1 = nc.dram_tensor("w1", [IN, HID], F32, kind="ExternalInput")
    w1a = nc.dram_tensor("w1a", [IN, 1], F32, kind="ExternalInput")
    w1d = nc.dram_tensor("w1d", [IN, 1], F32, kind="ExternalInput")
    haug = nc.dram_tensor("haug", [npad, ROWF], F32, kind="ExternalOutput")
    edo = nc.dram_tensor("edo", [128, ntiles], F32, kind="ExternalOutput")

    with tile.TileContext(nc) as tc:
        with (
            tc.tile_pool(name="sbuf", bufs=3) as pool,
            tc.tile_pool(name="cpool", bufs=1) as cpool,
            tc.tile_pool(name="psum", bufs=2, space="PSUM") as psum,
        ):
            w1_t = cpool.tile([128, IN // 128, HID], F32)
            nc.sync.dma_start(
                out=w1_t[:], in_=w1[:].rearrange("(a k) f -> k a f", k=128)
            )
            w1a_t = cpool.tile([128, IN // 128, 1], F32)
            nc.sync.dma_start(
                out=w1a_t[:], in_=w1a[:].rearrange("(a k) f -> k a f", k=128)
            )
            w1d_t = cpool.tile([128, IN // 128, 1], F32)
            nc.sync.dma_start(
                out=w1d_t[:], in_=w1d[:].rearrange("(a k) f -> k a f", k=128)
            )
            ed_sb = cpool.tile([128, ntiles], F32)

            for t in range(ntiles):
                xt = pool.tile([128, IN // 128, 128], F32, tag="xt")
                nc.sync.dma_start(
                    out=xt[:],
                    in_=xT[:, t * 128 : (t + 1) * 128].rearrange(
                        "(a k) n -> k a n", k=128
                    ),
                )
                hp = psum.tile([128, HID], F32, tag="hp")
                esp = psum.tile([128, 1], F32, tag="esp")
                edp = psum.tile([128, 1], F32, tag="edp")
                for a in range(IN // 128):
                    st = a == 0
                    sp = a == IN // 128 - 1
                    nc.tensor.matmul(
                        out=hp[:], lhsT=xt[:, a], rhs=w1_t[:, a], start=st, stop=sp
                    )
                    nc.tensor.matmul(
                        out=esp[:], lhsT=xt[:, a], rhs=w1a_t[:, a], start=st, stop=sp
                    )
                    nc.tensor.matmul(
                        out=edp[:], lhsT=xt[:, a], rhs=w1d_t[:, a], start=st, stop=sp
                    )
                ha = pool.tile([128, ROWF], F32, tag="ha")
                nc.scalar.activation(ha[:, 0:HID], hp[:], AF.Copy)
                nc.vector.tensor_copy(ha[:, HID : HID + 1], esp[:])
                nc.vector.tensor_copy(ed_sb[:, t : t + 1], edp[:])
                nc.sync.dma_start(
                    out=haug[t * 128 : (t + 1) * 128, :], in_=ha[:]
                )
            nc.sync.dma_start(out=edo[:], in_=ed_sb[:])
    nc.finalize()
    return nc


# ---------------------------------------------------------------- phase 2
def build_phase2(nchunks, groups):
    """nchunks: per-window chunk counts (uniform across cores).
    groups: list of (w_start, w_end) gather groups."""
    TOT = int(np.sum(nchunks))
    offs = np.zeros(len(nchunks) + 1, dtype=int)
    offs[1:] = np.cumsum(nchunks)

    nc = bacc.Bacc("TRN2", target_bir_lowering=True)
    haug = nc.dram_tensor("haug", [N + 1, ROWF], F32, kind="ExternalInput")
    idxs = nc.dram_tensor("idxs", [128, TOT], I32, kind="ExternalInput")
    edw = nc.dram_tensor("edw", [128, NW], F32, kind="ExternalInput")
    pcw = nc.dram_tensor("pcw", [128, NW], F32, kind="ExternalInput")
    w2 = nc.dram_tensor("w2", [HID, OUT], F32, kind="ExternalInput")
    y = nc.dram_tensor("y", [NW * 128, OUT], F32, kind="ExternalOutput")

    with tile.TileContext(nc) as tc:
        with (
            tc.tile_pool(name="gpool", bufs=2) as gpool,
            tc.tile_pool(name="spool", bufs=3) as spool,
            tc.tile_pool(name="cpool", bufs=1) as cpool,
            tc.tile_pool(name="psum", bufs=2, space="PSUM") as psum,
            tc.tile_pool(name="psum2", bufs=2, space="PSUM") as psum2,
        ):
            ident = cpool.tile([128, 128], F32)
            make_identity(nc, ident[:])
            w2_t = cpool.tile([HID, OUT], F32)
            nc.sync.dma_start(out=w2_t[:], in_=w2[:])
            edw_t = cpool.tile([128, NW], F32)
            nc.sync.dma_start(out=edw_t[:], in_=edw[:])
            pcw_t = cpool.tile([128, NW], F32)
            nc.sync.dma_start(out=pcw_t[:], in_=pcw[:])

            for (w0, w1_) in groups:
                c0, c1 = int(offs[w0]), int(offs[w1_])
                ncols = c1 - c0
                it = gpool.tile([128, ncols], I32, tag="it")
                nc.sync.dma_start(out=it[:], in_=idxs[:, c0:c1])
                gt = gpool.tile([128, ncols * ROWF], F32, tag="gt")
                # HW dynamic-offset DGE applies ONE offset per partition per
                # call (scalar_dynamic_offset level), so issue one indirect
                # DMA per idx column (128 rows per call).
                for cc in range(ncols):
                    nc.gpsimd.indirect_dma_start(
                        out=gt[:, cc * ROWF : (cc + 1) * ROWF],
                        out_offset=None,
                        in_=haug[:],
                        in_offset=bass.IndirectOffsetOnAxis(
                            ap=it[:, cc : cc + 1], axis=0
                        ),
                    )
                gt3 = gt[:].rearrange("p (c f) -> p c f", f=ROWF)
                for w in range(w0, w1_):
                    nch = int(nchunks[w])
                    lo = int(offs[w]) - c0
                    # alpha = sigmoid(es + ed); ex = exp(alpha)
                    alpha = spool.tile([128, nch], F32, tag="alpha")
                    nc.scalar.activation(
                        alpha[:],
                        gt3[:, lo : lo + nch, HID : HID + 1].rearrange(
                            "p c f -> p (c f)"
                        ),
                        AF.Sigmoid,
                        bias=edw_t[:, w : w + 1],
                    )
                    ex = spool.tile([128, nch], F32, tag="ex")
                    nc.scalar.activation(ex[:], alpha[:], AF.Exp)
                    # denominator
                    den = spool.tile([128, 1], F32, tag="den")
                    nc.vector.reduce_sum(
                        den[:], ex[:], axis=mybir.AxisListType.X
                    )
                    nc.vector.tensor_tensor(
                        out=den[:], in0=den[:], in1=pcw_t[:, w : w + 1],
                        op=ALU.subtract,
                    )
                    nc.vector.tensor_scalar_max(den[:], den[:], 0.5)
                    recip = spool.tile([128, 1], F32, tag="recip")
                    nc.vector.reciprocal(recip[:], den[:])
                    # scale all chunks by ex (broadcast along feature dim)
                    gs = spool.tile([128, nch * HID], F32, tag="gs")
                    nc.vector.tensor_tensor(
                        out=gs[:].rearrange("p (c f) -> p c f", f=HID),
                        in0=gt3[:, lo : lo + nch, 0:HID],
                        in1=ex[:, :, None].to_broadcast([128, nch, HID]),
                        op=ALU.mult,
                    )
                    acc = psum.tile([128, HID], F32, tag="acc")
                    for c in range(nch):
                        nc.tensor.matmul(
                            out=acc[:],
                            lhsT=ident[:],
                            rhs=gs[:, c * HID : (c + 1) * HID],
                            start=(c == 0),
                            stop=(c == nch - 1),
                        )
                    # ELU(acc * recip): x - relu(x) = min(x,0)
                    xs = spool.tile([128, HID], F32, tag="xs")
                    nc.vector.tensor_scalar(
                        out=xs[:], in0=acc[:], scalar1=recip[:],
                        scalar2=None, op0=ALU.mult,
                    )
                    mm = spool.tile([128, HID], F32, tag="mm")
                    nc.vector.tensor_scalar_min(mm[:], xs[:], 0.0)
                    ee = spool.tile([128, HID], F32, tag="ee")
                    nc.scalar.activation(ee[:], mm[:], AF.Exp)
                    rr = spool.tile([128, HID], F32, tag="rr")
                    nc.vector.tensor_scalar(
                        out=rr[:], in0=xs[:], scalar1=0.0, scalar2=-1.0,
                        op0=ALU.max, op1=ALU.add,
                    )
                    h1 = spool.tile([128, HID], F32, tag="h1")
                    nc.vector.tensor_tensor(
                        out=h1[:], in0=rr[:], in1=ee[:], op=ALU.add
                    )
                    # y_w = h1 @ W2  (transpose h1 on PE, then matmul)
                    h1tp = psum2.tile([128, HID], F32, tag="h1tp")
                    nc.tensor.transpose(
                        out=h1tp[:], in_=h1[:], identity=ident[:]
                    )
                    h1t = spool.tile([128, HID], F32, tag="h1t")
                    nc.scalar.activation(h1t[:], h1tp[:], AF.Copy)
                    yp = psum2.tile([128, OUT], F32, tag="yp")
                    nc.tensor.matmul(
                        out=yp[:], lhsT=h1t[:], rhs=w2_t[:],
                        start=True, stop=True,
                    )
                    yt = spool.tile([128, OUT], F32, tag="yt")
                    nc.scalar.activation(yt[:], yp[:], AF.Copy)
                    nc.sync.dma_start(
                        out=y[w * 128 : (w + 1) * 128, :], in_=yt[:]
                    )
    nc.finalize()
    return nc


# ---------------------------------------------------------------- host glue
def kernel(x, edge_index, W1, att_src, att_dst, W2):
    x = np.asarray(x, dtype=np.float32)
    edge_index = np.asarray(edge_index)
    W1 = np.asarray(W1, dtype=np.float32)
    att_src = np.asarray(att_src, dtype=np.float32)
    att_dst = np.asarray(att_dst, dtype=np.float32)
    W2 = np.asarray(W2, dtype=np.float32)

    src = edge_index[0].astype(np.int64)
    dst = edge_index[1].astype(np.int64)

    # ---- phase 1: sharded h/es/ed compute
    xT = np.ascontiguousarray(x.T)  # [IN, N]
    w1a = (W1 @ att_src).reshape(IN, 1).astype(np.float32)
    w1d = (W1 @ att_dst).reshape(IN, 1).astype(np.float32)
    ntiles = (NPC + 127) // 128
    npad = ntiles * 128

    nc1 = build_phase1()
    in_maps1 = []
    for c in range(NCORES):
        sh = xT[:, c * NPC : (c + 1) * NPC]
        if sh.shape[1] < npad:
            sh = np.concatenate(
                [sh, np.zeros((IN, npad - sh.shape[1]), np.float32)], axis=1
            )
        in_maps1.append(
            {"xT": np.ascontiguousarray(sh), "w1": W1, "w1a": w1a, "w1d": w1d}
        )
    trace = os.environ.get("BASS_GAT_TRACE") == "1"
    tkw = dict(trace=True, trace_cores=[0]) if trace else {}
    if trace:
        _patch_perfetto()
    t0 = time.time()
    res1 = run_bass_kernel_spmd(nc1, in_maps1, core_ids=list(range(NCORES)), **tkw)
    _timings["phase1_wall"] = time.time() - t0
    _timings["phase1_ns"] = res1.exec_time_ns

    haug_full = np.zeros((N + 1, ROWF), np.float32)
    ed_full = np.zeros(N, np.float32)
    for c in range(NCORES):
        haug_full[c * NPC : (c + 1) * NPC] = res1.results[c]["haug"][:NPC]
        ed_full[c * NPC : (c + 1) * NPC] = (
            res1.results[c]["edo"].T.ravel()[:NPC]
        )
    haug_full[N, HID] = -1e30  # dummy row: es=-inf, h=0

    # ---- host edge routing: per-core degree-sorted windows
    deg = np.bincount(dst, minlength=N)
    orders = []
    nch_per_core = np.zeros((NCORES, NW), np.int64)
    for c in range(NCORES):
        dl = deg[c * NPC : (c + 1) * NPC]
        order = np.argsort(-dl, kind="stable")
        orders.append(order)
        dls = dl[order]
        for w in range(NW):
            j0 = w * 128
            nch_per_core[c, w] = dls[j0] if j0 < NPC else 0
    nchunks = np.maximum(nch_per_core.max(axis=0), 1)
    offs = np.zeros(NW + 1, dtype=np.int64)
    offs[1:] = np.cumsum(nchunks)
    TOT = int(offs[-1])

    # gather groups
    groups = []
    w0 = 0
    while w0 < NW:
        w1_ = w0 + 1
        while w1_ < NW and offs[w1_ + 1] - offs[w0] <= GCOLS:
            w1_ += 1
        groups.append((w0, w1_))
        w0 = w1_

    # per-core idx/padcnt/ed arrays
    eorder = np.argsort(dst, kind="stable")
    src_s = src[eorder]
    estarts = np.zeros(N + 1, np.int64)
    estarts[1:] = np.cumsum(deg)

    in_maps2 = []
    for c in range(NCORES):
        order = orders[c]
        rank = np.empty(NPC, np.int64)
        rank[order] = np.arange(NPC)
        idx_arr = np.full((128, TOT), DUMMY, np.int32)
        padcnt = np.zeros((128, NW), np.float32)
        edw = np.zeros((128, NW), np.float32)
        for wloc in range(NW):
            j0 = wloc * 128
            nodes = order[j0 : j0 + 128]  # local ids, len<=128
            for p, j in enumerate(nodes):
                g = c * NPC + j
                d = deg[g]
                s0 = estarts[g]
                cols = slice(int(offs[wloc]), int(offs[wloc]) + int(d))
                idx_arr[p, cols] = src_s[s0 : s0 + d]
                padcnt[p, wloc] = nchunks[wloc] - d
                edw[p, wloc] = ed_full[g]
            for p in range(len(nodes), 128):
                padcnt[p, wloc] = nchunks[wloc]
        in_maps2.append(
            {
                "haug": haug_full,
                "idxs": idx_arr,
                "edw": edw,
                "pcw": padcnt,
                "w2": W2,
            }
        )

    nc2 = build_phase2(nchunks, groups)
    t0 = time.time()
    res2 = run_bass_kernel_spmd(nc2, in_maps2, core_ids=list(range(NCORES)), **tkw)
    _timings["phase2_wall"] = time.time() - t0
    _timings["phase2_ns"] = res2.exec_time_ns

    out = np.zeros((N, OUT), np.float32)
    for c in range(NCORES):
        yv = res2.results[c]["y"]
        order = orders[c]
        valid = min(NPC, NW * 128)
        out[c * NPC + order[:valid]] = yv[:valid]
    return out


# revision 5
# speedup vs baseline: 1.0059x; 1.0059x over previous
"""GAT encoder (gnn_message_passing) on 8 trn2 NeuronCores via Bass.

Strategy (graph-parallel, dst-sharded):
  Launch 1 (sharded by node range): h = x@W1, es = x@(W1@att_src),
    ed = x@(W1@att_dst). Each core writes haug rows [h fp32 | es fp32]
    (129 floats = 516B) for its 6250 nodes, plus ed.
  Host: concatenate haug shards (+1 dummy row with es=-1e30), permute ed
    into degree-sorted window layout, route edges to dst-owner cores.
  Launch 2 (per core, dst windows of 128 degree-sorted nodes): indirect-DMA
    gather of haug rows for all edge slots (padded to per-window uniform
    chunk count); alpha = sigmoid(es_src + ed_dst); ex = exp(alpha)
    (max-subtraction dropped: alpha in (0,1) so exp is stable; softmax
    weights are mathematically identical); per-chunk scale rows by ex and
    accumulate via identity-stationary matmuls into PSUM; denominator =
    free-dim reduce of ex minus pad count; out = ELU(acc/den) @ W2.
"""
import os
import sys
import time

sys.path.insert(0, "/opt/trn_rl_repo")

import numpy as np

N, E = 50000, 800000
IN, HID, OUT = 256, 128, 128
NCORES = 8
NPC = N // NCORES            # nodes per core
NW = NPC // 128              # windows per core (49 when NPC=6272... 6250/128)
assert NPC % 2 == 0
NW = (NPC + 127) // 128      # 49 windows; last window partial (6250 = 48*128+106)
ROWF = HID + 1               # floats per haug row (h + es)
DUMMY = N                    # dummy haug row index (es = -1e30)
GCOLS = 32                   # max idx columns per gather call

_timings = {}


def _patch_env():
    """Tile/perfetto compatibility patches for this container."""
    import concourse.tile as tile
    from concourse.tile import ScopedClock

    def _drain_and_barrier_split(self, tick_clock, wait_clock):
        nc = self.nc
        probe = nc.sync.nop()
        wait_clock.add_sem_waits(
            probe.ins, ScopedClock({None: tick_clock.global_clock})
        )
        waits = list(probe.ins.sync_info.on_wait or [])
        probe.ins.sync_info.on_wait = []
        from concourse import mybir

        for w in waits:
            inst = nc.sync.nop()
            if inst.ins.sync_info is None:
                inst.ins.sync_info = mybir.SyncInfo(on_wait=[w], on_update=[])
            else:
                inst.ins.sync_info.on_wait = [w]
        nc.sync.drain()
        nc.all_engine_barrier()
        assert self.sems is not None
        popped = nc._tile_sem_poison_stack.pop()
        assert popped is self._sem_poison
        nc.clear_and_free_semaphores(list(self.sems.allocated().values()))
        nc.all_engine_barrier()

    tile.TileContext._drain_and_barrier = _drain_and_barrier_split


_patch_env()


def _patch_perfetto():
    try:
        from gauge import trn_perfetto

        cls = trn_perfetto.TrnPerfettoConv
        if not getattr(cls, "_no_hlo_patched", False):
            _orig_init = cls.__init__

            def _init_no_hlo(self, *a, **k):
                k["annotate_hlo"] = False
                if len(a) >= 2:
                    a = (a[0], False) + a[2:]
                _orig_init(self, *a, **k)

            cls.__init__ = _init_no_hlo
            cls._no_hlo_patched = True
    except Exception:
        pass


import concourse.bass as bass
import concourse.bacc as bacc
import concourse.tile as tile
from concourse import mybir
from concourse.bass_utils import run_bass_kernel_spmd
from concourse.masks import make_identity

F32 = mybir.dt.float32
I32 = mybir.dt.int32
AF = mybir.ActivationFunctionType
ALU = mybir.AluOpType


# ---------------------------------------------------------------- phase 1
def build_phase1():
    nc = bacc.Bacc("TRN2", target_bir_lowering=True)
    ntiles = (NPC + 127) // 128
    npad = ntiles * 128
    xT = nc.dram_tensor("xT", [IN, npad], F32, kind="ExternalInput")
    w1 = nc.dram_tensor("w1", [IN, HID], F32, kind="ExternalInput")
    w1a = nc.dram_tensor("w1a", [IN, 1], F32, kind="ExternalInput")
    w1d = nc.dram_tensor("w1d", [IN, 1], F32, kind="ExternalInput")
    haug = nc.dram_tensor("haug", [npad, ROWF], F32, kind="ExternalOutput")
    edo = nc.dram_tensor("edo", [128, ntiles], F32, kind="ExternalOutput")

    with tile.TileContext(nc) as tc:
        with (
            tc.tile_pool(name="sbuf", bufs=3) as pool,
            tc.tile_pool(name="cpool", bufs=1) as cpool,
            tc.tile_pool(name="psum", bufs=2, space="PSUM") as psum,
        ):
            w1_t = cpool.tile([128, IN // 128, HID], F32)
            nc.sync.dma_start(
                out=w1_t[:], in_=w1[:].rearrange("(a k) f -> k a f", k=128)
            )
            w1a_t = cpool.tile([128, IN // 128, 1], F32)
            nc.sync.dma_start(
                out=w1a_t[:], in_=w1a[:].rearrange("(a k) f -> k a f", k=128)
            )
            w1d_t = cpool.tile([128, IN // 128, 1], F32)
            nc.sync.dma_start(
                out=w1d_t[:], in_=w1d[:].rearrange("(a k) f -> k a f", k=128)
            )
            ed_sb = cpool.tile([128, ntiles], F32)

            for t in range(ntiles):
                xt = pool.tile([128, IN // 128, 128], F32, tag="xt")
                nc.sync.dma_start(
                    out=xt[:],
                    in_=xT[:, t * 128 : (t + 1) * 128].rearrange(
                        "(a k) n -> k a n", k=128
                    ),
                )
                hp = psum.tile([128, HID], F32, tag="hp")
                esp = psum.tile([128, 1], F32, tag="esp")
                edp = psum.tile([128, 1], F32, tag="edp")
                for a in range(IN // 128):
                    st = a == 0
                    sp = a == IN // 128 - 1
                    nc.tensor.matmul(
                        out=hp[:], lhsT=xt[:, a], rhs=w1_t[:, a], start=st, stop=sp
                    )
                    nc.tensor.matmul(
                        out=esp[:], lhsT=xt[:, a], rhs=w1a_t[:, a], start=st, stop=sp
                    )
                    nc.tensor.matmul(
                        out=edp[:], lhsT=xt[:, a], rhs=w1d_t[:, a], start=st, stop=sp
                    )
                ha = pool.tile([128, ROWF], F32, tag="ha")
                nc.scalar.activation(ha[:, 0:HID], hp[:], AF.Copy)
                nc.vector.tensor_copy(ha[:, HID : HID + 1], esp[:])
                nc.vector.tensor_copy(ed_sb[:, t : t + 1], edp[:])
                nc.sync.dma_start(
                    out=haug[t * 128 : (t + 1) * 128, :], in_=ha[:]
                )
            nc.sync.dma_start(out=edo[:], in_=ed_sb[:])
    nc.finalize()
    return nc


# ---------------------------------------------------------------- phase 2
def build_phase2(nchunks, groups):
    """nchunks: per-window chunk counts (uniform across cores).
    groups: list of (w_start, w_end) gather groups."""
    TOT = int(np.sum(nchunks))
    offs = np.zeros(len(nchunks) + 1, dtype=int)
    offs[1:] = np.cumsum(nchunks)

    nc = bacc.Bacc("TRN2", target_bir_lowering=True)
    haug = nc.dram_tensor("haug", [N + 1, ROWF], F32, kind="ExternalInput")
    idxs = nc.dram_tensor("idxs", [128, TOT], I32, kind="ExternalInput")
    edw = nc.dram_tensor("edw", [128, NW], F32, kind="ExternalInput")
    pcw = nc.dram_tensor("pcw", [128, NW], F32, kind="ExternalInput")
    w2 = nc.dram_tensor("w2", [HID, OUT], F32, kind="ExternalInput")
    y = nc.dram_tensor("y", [NW * 128, OUT], F32, kind="ExternalOutput")

    with tile.TileContext(nc) as tc:
        with (
            tc.tile_pool(name="gpool", bufs=3) as gpool,
            tc.tile_pool(name="spool", bufs=6) as spool,
            tc.tile_pool(name="cpool", bufs=1) as cpool,
            tc.tile_pool(name="psum", bufs=2, space="PSUM") as psum,
            tc.tile_pool(name="psum2", bufs=2, space="PSUM") as psum2,
        ):
            ident = cpool.tile([128, 128], F32)
            make_identity(nc, ident[:])
            w2_t = cpool.tile([HID, OUT], F32)
            nc.sync.dma_start(out=w2_t[:], in_=w2[:])
            edw_t = cpool.tile([128, NW], F32)
            nc.sync.dma_start(out=edw_t[:], in_=edw[:])
            pcw_t = cpool.tile([128, NW], F32)
            nc.sync.dma_start(out=pcw_t[:], in_=pcw[:])

            for (w0, w1_) in groups:
                c0, c1 = int(offs[w0]), int(offs[w1_])
                ncols = c1 - c0
                it = gpool.tile([128, ncols], I32, tag="it")
                nc.sync.dma_start(out=it[:], in_=idxs[:, c0:c1])
                gt = gpool.tile([128, ncols * ROWF], F32, tag="gt")
                # HW dynamic-offset DGE applies ONE offset per partition per
                # call (scalar_dynamic_offset level), so issue one indirect
                # DMA per idx column (128 rows per call).
                for cc in range(ncols):
                    nc.gpsimd.indirect_dma_start(
                        out=gt[:, cc * ROWF : (cc + 1) * ROWF],
                        out_offset=None,
                        in_=haug[:],
                        in_offset=bass.IndirectOffsetOnAxis(
                            ap=it[:, cc : cc + 1], axis=0
                        ),
                    )
                gt3 = gt[:].rearrange("p (c f) -> p c f", f=ROWF)
                for w in range(w0, w1_):
                    nch = int(nchunks[w])
                    lo = int(offs[w]) - c0
                    # alpha = sigmoid(es + ed); ex = exp(alpha)
                    alpha = spool.tile([128, nch], F32, tag="alpha")
                    nc.scalar.activation(
                        alpha[:],
                        gt3[:, lo : lo + nch, HID : HID + 1].rearrange(
                            "p c f -> p (c f)"
                        ),
                        AF.Sigmoid,
                        bias=edw_t[:, w : w + 1],
                    )
                    ex = spool.tile([128, nch], F32, tag="ex")
                    nc.scalar.activation(ex[:], alpha[:], AF.Exp)
                    # denominator
                    den = spool.tile([128, 1], F32, tag="den")
                    nc.vector.reduce_sum(
                        den[:], ex[:], axis=mybir.AxisListType.X
                    )
                    nc.vector.tensor_tensor(
                        out=den[:], in0=den[:], in1=pcw_t[:, w : w + 1],
                        op=ALU.subtract,
                    )
                    nc.vector.tensor_scalar_max(den[:], den[:], 0.5)
                    recip = spool.tile([128, 1], F32, tag="recip")
                    nc.vector.reciprocal(recip[:], den[:])
                    # scale all chunks by ex (broadcast along feature dim)
                    gs = spool.tile([128, nch * HID], F32, tag="gs")
                    nc.vector.tensor_tensor(
                        out=gs[:].rearrange("p (c f) -> p c f", f=HID),
                        in0=gt3[:, lo : lo + nch, 0:HID],
                        in1=ex[:, :, None].to_broadcast([128, nch, HID]),
                        op=ALU.mult,
                    )
                    acc = psum.tile([128, HID], F32, tag="acc")
                    for c in range(nch):
                        nc.tensor.matmul(
                            out=acc[:],
                            lhsT=ident[:],
                            rhs=gs[:, c * HID : (c + 1) * HID],
                            start=(c == 0),
                            stop=(c == nch - 1),
                        )
                    # ELU(acc * recip): x - relu(x) = min(x,0)
                    xs = spool.tile([128, HID], F32, tag="xs")
                    nc.vector.tensor_scalar(
                        out=xs[:], in0=acc[:], scalar1=recip[:],
                        scalar2=None, op0=ALU.mult,
                    )
                    mm = spool.tile([128, HID], F32, tag="mm")
                    nc.vector.tensor_scalar_min(mm[:], xs[:], 0.0)
                    ee = spool.tile([128, HID], F32, tag="ee")
                    nc.scalar.activation(ee[:], mm[:], AF.Exp)
                    rr = spool.tile([128, HID], F32, tag="rr")
                    nc.vector.tensor_scalar(
                        out=rr[:], in0=xs[:], scalar1=0.0, scalar2=-1.0,
                        op0=ALU.max, op1=ALU.add,
                    )
                    h1 = spool.tile([128, HID], F32, tag="h1")
                    nc.vector.tensor_tensor(
                        out=h1[:], in0=rr[:], in1=ee[:], op=ALU.add
                    )
                    # y_w = h1 @ W2  (transpose h1 on PE, then matmul)
                    h1tp = psum2.tile([128, HID], F32, tag="h1tp")
                    nc.tensor.transpose(
                        out=h1tp[:], in_=h1[:], identity=ident[:]
                    )
                    h1t = spool.tile([128, HID], F32, tag="h1t")
                    nc.scalar.activation(h1t[:], h1tp[:], AF.Copy)
                    yp = psum2.tile([128, OUT], F32, tag="yp")
                    nc.tensor.matmul(
                        out=yp[:], lhsT=h1t[:], rhs=w2_t[:],
                        start=True, stop=True,
                    )
                    yt = spool.tile([128, OUT], F32, tag="yt")
                    nc.scalar.activation(yt[:], yp[:], AF.Copy)
                    nc.sync.dma_start(
                        out=y[w * 128 : (w + 1) * 128, :], in_=yt[:]
                    )
    nc.finalize()
    return nc


# ---------------------------------------------------------------- host glue
def kernel(x, edge_index, W1, att_src, att_dst, W2):
    x = np.asarray(x, dtype=np.float32)
    edge_index = np.asarray(edge_index)
    W1 = np.asarray(W1, dtype=np.float32)
    att_src = np.asarray(att_src, dtype=np.float32)
    att_dst = np.asarray(att_dst, dtype=np.float32)
    W2 = np.asarray(W2, dtype=np.float32)

    src = edge_index[0].astype(np.int64)
    dst = edge_index[1].astype(np.int64)

    # ---- phase 1: sharded h/es/ed compute
    xT = np.ascontiguousarray(x.T)  # [IN, N]
    w1a = (W1 @ att_src).reshape(IN, 1).astype(np.float32)
    w1d = (W1 @ att_dst).reshape(IN, 1).astype(np.float32)
    ntiles = (NPC + 127) // 128
    npad = ntiles * 128

    nc1 = build_phase1()
    in_maps1 = []
    for c in range(NCORES):
        sh = xT[:, c * NPC : (c + 1) * NPC]
        if sh.shape[1] < npad:
            sh = np.concatenate(
                [sh, np.zeros((IN, npad - sh.shape[1]), np.float32)], axis=1
            )
        in_maps1.append(
            {"xT": np.ascontiguousarray(sh), "w1": W1, "w1a": w1a, "w1d": w1d}
        )
    trace = os.environ.get("BASS_GAT_TRACE") == "1"
    tkw = dict(trace=True, trace_cores=[0]) if trace else {}
    if trace:
        _patch_perfetto()
    t0 = time.time()
    res1 = run_bass_kernel_spmd(nc1, in_maps1, core_ids=list(range(NCORES)), **tkw)
    _timings["phase1_wall"] = time.time() - t0
    _timings["phase1_ns"] = res1.exec_time_ns

    haug_full = np.zeros((N + 1, ROWF), np.float32)
    ed_full = np.zeros(N, np.float32)
    for c in range(NCORES):
        haug_full[c * NPC : (c + 1) * NPC] = res1.results[c]["haug"][:NPC]
        ed_full[c * NPC : (c + 1) * NPC] = (
            res1.results[c]["edo"].T.ravel()[:NPC]
        )
    haug_full[N, HID] = -1e30  # dummy row: es=-inf, h=0

    # ---- host edge routing: per-core degree-sorted windows
    deg = np.bincount(dst, minlength=N)
    orders = []
    nch_per_core = np.zeros((NCORES, NW), np.int64)
    for c in range(NCORES):
        dl = deg[c * NPC : (c + 1) * NPC]
        order = np.argsort(-dl, kind="stable")
        orders.append(order)
        dls = dl[order]
        for w in range(NW):
            j0 = w * 128
            nch_per_core[c, w] = dls[j0] if j0 < NPC else 0
    nchunks = np.maximum(nch_per_core.max(axis=0), 1)
    offs = np.zeros(NW + 1, dtype=np.int64)
    offs[1:] = np.cumsum(nchunks)
    TOT = int(offs[-1])

    # gather groups
    groups = []
    w0 = 0
    while w0 < NW:
        w1_ = w0 + 1
        while w1_ < NW and offs[w1_ + 1] - offs[w0] <= GCOLS:
            w1_ += 1
        groups.append((w0, w1_))
        w0 = w1_

    # per-core idx/padcnt/ed arrays
    eorder = np.argsort(dst, kind="stable")
    src_s = src[eorder]
    estarts = np.zeros(N + 1, np.int64)
    estarts[1:] = np.cumsum(deg)

    in_maps2 = []
    for c in range(NCORES):
        order = orders[c]
        rank = np.empty(NPC, np.int64)
        rank[order] = np.arange(NPC)
        idx_arr = np.full((128, TOT), DUMMY, np.int32)
        padcnt = np.zeros((128, NW), np.float32)
        edw = np.zeros((128, NW), np.float32)
        for wloc in range(NW):
            j0 = wloc * 128
            nodes = order[j0 : j0 + 128]  # local ids, len<=128
            for p, j in enumerate(nodes):
                g = c * NPC + j
                d = deg[g]
                s0 = estarts[g]
                cols = slice(int(offs[wloc]), int(offs[wloc]) + int(d))
                idx_arr[p, cols] = src_s[s0 : s0 + d]
                padcnt[p, wloc] = nchunks[wloc] - d
                edw[p, wloc] = ed_full[g]
            for p in range(len(nodes), 128):
                padcnt[p, wloc] = nchunks[wloc]
        in_maps2.append(
            {
                "haug": haug_full,
                "idxs": idx_arr,
                "edw": edw,
                "pcw": padcnt,
                "w2": W2,
            }
        )

    nc2 = build_phase2(nchunks, groups)
    t0 = time.time()
    res2 = run_bass_kernel_spmd(nc2, in_maps2, core_ids=list(range(NCORES)), **tkw)
    _timings["phase2_wall"] = time.time() - t0
    _timings["phase2_ns"] = res2.exec_time_ns

    out = np.zeros((N, OUT), np.float32)
    for c in range(NCORES):
        yv = res2.results[c]["y"]
        order = orders[c]
        valid = min(NPC, NW * 128)
        out[c * NPC + order[:valid]] = yv[:valid]
    return out
